# revision 13
# baseline (speedup 1.0000x reference)
"""LocalAttention Trainium2 Bass kernel (bf16, banded, flipped-PV).

Problem: x (2, 2048, 1024) f32 -> QKV proj (16 heads, d=64), local-window
attention (|i-j| <= 128), output projection.

Sharding (no collectives): 8 shards = 2 batches x 4 seq-chunks of 512 q rows.
Each core gets a 768-row KV halo slice [qs-128, qs+640), zero-padded at the
sequence edges, so q row i sits at kv row i+128 on every core.

Attention is banded with 128-row q tiles: q tile t's window covers exactly kv
tiles t, t+1, t+2 of the local 6-tile kv range. Tile t+1 is always fully
in-window (no mask); tiles t and t+2 get canonical triangular masks (edge
validity folded in per core); the remaining 3 kv tiles are skipped entirely.

Per (head, q-tile) the PV matmul runs "flipped" -- out [128 q, 64 v + 1 ones]
-- so the softmax denominator lands per-partition: a [128,1] reciprocal and a
per-partition tensor_scalar_mul normalize it. Head pairs are then transposed
back to [head*64, q] via the PE (identity matmul) for the output projection.

All matmul operands are bf16 (f32 PSUM accumulation). Engine split: PE
matmuls/transposes; Scalar exp only; GpSimd proj copies + half the mask muls;
Vector the rest. The emission order software-pipelines head pairs so the PE
stays continuously busy (p-state ramp) while Scalar/Vector chew on the
previous head's softmax.
"""

from contextlib import ExitStack

import numpy as np
import ml_dtypes

import concourse.bacc as bacc
import concourse.mybir as mybir
import concourse.tile as tile
from concourse.bass_utils import run_bass_kernel_spmd

F32 = mybir.dt.float32
BF16 = mybir.dt.bfloat16
AF = mybir.ActivationFunctionType
BF16_NP = ml_dtypes.bfloat16

EMB = 1024
NHEAD = 16
DHEAD = 64
WIN = 128
BATCH = 2
SEQ = 2048
NQ = 512            # q rows per core
NKV = 768           # kv rows per core (q rows + clamped 128 halo each side)
NT_E = EMB // 128   # 8 e-tiles
NT_KV = NKV // 128  # 6 kv-tiles
NT_Q = NQ // 128    # 4 q-tiles of 128
QOFF = 128          # q row i == kv row i + QOFF
SCALE = 1.0 / np.sqrt(EMB)

LAST_RESULT = None  # BassKernelResults of the most recent run (for profiling)


def _body(ctx, tc, aps):
    nc = tc.nc

    pbig = ctx.enter_context(tc.tile_pool(name="big", bufs=1))
    pqt = ctx.enter_context(tc.tile_pool(name="qt", bufs=3))
    pkt = ctx.enter_context(tc.tile_pool(name="kt", bufs=3))
    pv = ctx.enter_context(tc.tile_pool(name="v", bufs=6))
    pset = ctx.enter_context(tc.tile_pool(name="set", bufs=12))
    pov = ctx.enter_context(tc.tile_pool(name="ov", bufs=4))
    pot = ctx.enter_context(tc.tile_pool(name="ot", bufs=8))
    pfin = ctx.enter_context(tc.tile_pool(name="fin", bufs=2))
    psml = ctx.enter_context(tc.tile_pool(name="small", bufs=1))
    prc = ctx.enter_context(tc.tile_pool(name="rc", bufs=6))
    pps = ctx.enter_context(tc.tile_pool(name="ps", bufs=2, space="PSUM"))
    pqk = ctx.enter_context(tc.tile_pool(name="qk", bufs=2, space="PSUM"))
    ppv = ctx.enter_context(tc.tile_pool(name="pv", bufs=3, space="PSUM"))
    ppt = ctx.enter_context(tc.tile_pool(name="pt", bufs=1, space="PSUM"))

    mm = nc.tensor.matmul

    # ---- persistent tiles ----
    xb = pbig.tile([128, NT_E * NKV], BF16, tag="xb")
    wqt = pbig.tile([128, EMB * NT_E // 8 * 8], BF16, tag="wq", name="wqt")
    wkt = pbig.tile([128, 8192], BF16, tag="wk", name="wkt")
    wvn = pbig.tile([128, 8192], BF16, tag="wv", name="wvn")
    won = pbig.tile([128, 8192], BF16, tag="wo", name="won")
    mid = pbig.tile([128, 9 * 128], BF16, tag="mid", name="mid")
    bqs = psml.tile([128, NT_E], F32, tag="bq", name="bqs")
    bks = psml.tile([128, NT_E], F32, tag="bk", name="bks")
    bvs = psml.tile([128, NT_E], F32, tag="bv", name="bvs")
    bob = psml.tile([128, EMB], F32, tag="bo", name="bob")
    wrm = psml.tile([1, 1], F32, tag="wrm", name="wrm")

    # warm the Exp activation table before the first real softmax needs it
    nc.gpsimd.memset(wrm[:], 0.0)
    nc.scalar.activation(wrm[:], wrm[:], AF.Exp, scale=1.0)

    # ---- input DMAs ----
    # Critical path first: Q-proj(0) needs wq block 0 + all x blocks. Dispatch
    # wq0/wk0 on the GpSimd DGE queue (in parallel with x on Sync) so the
    # first matmul can start early; everything else streams behind x.
    nc.gpsimd.dma_start(wqt[:, 0:1024], aps["wq"][:, 0:1024])
    nc.gpsimd.dma_start(wkt[:, 0:1024], aps["wk"][:, 0:1024])
    for k in range(NT_E):  # x blocks split across two DGE queues
        eng = nc.sync if k % 2 == 0 else nc.scalar
        eng.dma_start(xb[:, k * NKV:(k + 1) * NKV],
                      aps["xb"][:, k * NKV:(k + 1) * NKV])
    nc.scalar.dma_start(mid[:], aps["mid"][:])
    nc.scalar.dma_start(bqs[:], aps["bqc"][:])
    nc.scalar.dma_start(bks[:], aps["bkc"][:])
    nc.scalar.dma_start(bvs[:], aps["bvc"][:])
    nc.scalar.dma_start(bob[:], aps["bob"][:])
    for k in range(NT_E):
        nc.sync.dma_start(wvn[:, k * 1024:(k + 1) * 1024],
                          aps["wv"][:, k * 1024:(k + 1) * 1024])
    for t in range(1, 8):
        nc.sync.dma_start(wqt[:, t * 1024:(t + 1) * 1024],
                          aps["wq"][:, t * 1024:(t + 1) * 1024])
        nc.sync.dma_start(wkt[:, t * 1024:(t + 1) * 1024],
                          aps["wk"][:, t * 1024:(t + 1) * 1024])
    for k in range(NT_E):
        nc.sync.dma_start(won[:, k * 1024:(k + 1) * 1024],
                          aps["wo"][:, k * 1024:(k + 1) * 1024])

    ident = mid[:, 8 * 128:9 * 128]

    # ---- v tiles (with ones column per head for the softmax denominator) ----
    v = []
    for j in range(NT_KV):
        vt = pv.tile([128, NHEAD * (DHEAD + 1)], BF16, tag="v", name=f"v{j}")
        vr = vt[:].rearrange("p (h d) -> p h d", d=DHEAD + 1)
        nc.gpsimd.memset(vr[:, :, DHEAD:DHEAD + 1], 1.0)
        v.append(vt)

    qt = [None] * NT_E
    kt = [None] * NT_E
    ot = [pot.tile([128, NQ], BF16, tag="ot", name=f"ot{i}") for i in range(NT_E)]
    sets = {}  # h -> list of (set_tile, tmin) per kv tile j
    ovs = {}   # t -> ov tile for the current head pair

    def emit_proj(th):
        # wq/wk are t-major: block th holds [k, 128] col-tiles contiguously.
        ps = pps.tile([128, NQ], F32, tag="ps", name=f"qp{th}")
        for k in range(NT_E):
            mm(ps[:], wqt[:, th * 1024 + k * 128:th * 1024 + (k + 1) * 128],
               xb[:, k * NKV + QOFF:k * NKV + QOFF + NQ],
               start=(k == 0), stop=(k == NT_E - 1))
        qt[th] = pqt.tile([128, NQ], BF16, tag="qt", name=f"qt{th}")
        nc.scalar.activation(qt[th][:], ps[:], AF.Identity,
                             bias=bqs[:, th:th + 1])
        kt[th] = pkt.tile([128, NKV], BF16, tag="kt", name=f"kt{th}")
        for half in range(2):
            ps = pps.tile([128, NQ], F32, tag="ps", name=f"kp{th}_{half}")
            for k in range(NT_E):
                mm(ps[:, 0:384],
                   wkt[:, th * 1024 + k * 128:th * 1024 + (k + 1) * 128],
                   xb[:, k * NKV + half * 384:k * NKV + (half + 1) * 384],
                   start=(k == 0), stop=(k == NT_E - 1))
            nc.vector.tensor_scalar_add(kt[th][:, half * 384:(half + 1) * 384],
                                        ps[:, 0:384], bks[:, th:th + 1])

    def emit_qk(h):
        th, off = h // 2, (h % 2) * DHEAD
        hs = []
        for j in range(NT_KV):
            tmin, tmax = max(0, j - 2), min(3, j)
            w = (tmax - tmin + 1) * 128
            qk = pqk.tile([128, 384], F32, tag="qk", name=f"qk{h}_{j}")
            mm(qk[:, 0:w], kt[th][off:off + DHEAD, j * 128:(j + 1) * 128],
               qt[th][off:off + DHEAD, tmin * 128:(tmax + 1) * 128],
               start=True, stop=True)
            st = pset.tile([128, 384], BF16, tag="set", name=f"set{h}_{j}")
            nc.scalar.activation(st[:, 0:w], qk[:, 0:w], AF.Exp,
                                 scale=float(SCALE))
            if j <= 3:  # lower-edge mask block at t == j
                eng = nc.gpsimd
                c0 = (j - tmin) * 128
                eng.tensor_mul(st[:, c0:c0 + 128], st[:, c0:c0 + 128],
                               mid[:, j * 128:(j + 1) * 128])
            if j >= 2:  # upper-edge mask block at t == j - 2
                eng = nc.gpsimd
                c0 = (j - 2 - tmin) * 128
                eng.tensor_mul(st[:, c0:c0 + 128], st[:, c0:c0 + 128],
                               mid[:, (j + 2) * 128:(j + 3) * 128])
            hs.append((st, tmin))
        sets[h] = hs

    def emit_pv(h):
        th, off = h // 2, (h % 2) * DHEAD
        for t in range(NT_Q):
            pvp = ppv.tile([128, DHEAD + 1], F32, tag="pv", name=f"pv{h}_{t}")
            for i, j in enumerate(range(t, t + 3)):
                st, tmin = sets[h][j]
                c0 = (t - tmin) * 128
                mm(pvp[:], st[:, c0:c0 + 128],
                   v[j][:, h * (DHEAD + 1):(h + 1) * (DHEAD + 1)],
                   start=(i == 0), stop=(i == 2))
            rc = prc.tile([128, 1], F32, tag="rc", name=f"rc{h}_{t}")
            nc.vector.reciprocal(rc[:], pvp[:, DHEAD:DHEAD + 1])
            if h % 2 == 0:
                ovs[t] = pov.tile([128, 128], BF16, tag="ov", name=f"ov{h}_{t}")
            nc.vector.tensor_scalar_mul(ovs[t][:, off:off + DHEAD],
                                        pvp[:, 0:DHEAD], rc[:])
        del sets[h]

    def emit_t(th):
        tp = ppt.tile([128, 512], BF16, tag="tp", name=f"tp{th}")
        for t in range(NT_Q):
            nc.tensor.transpose(tp[:, t * 128:(t + 1) * 128], ovs[t][:], ident)
        nc.vector.tensor_scalar_add(ot[th][:], tp[:], bvs[:, th:th + 1])

    # ---- emission: prologue, then the software-pipelined head-pair loop ----
    emit_proj(0)
    emit_qk(0)
    emit_qk(1)

    for j in range(NT_KV):  # V projection (kv-tile major)
        vr = v[j][:].rearrange("p (h d) -> p h d", d=DHEAD + 1)
        for half in range(2):
            ps = pps.tile([128, NQ], F32, tag="ps", name=f"vp{j}_{half}")
            for k in range(NT_E):
                mm(ps[:], xb[:, k * NKV + j * 128:k * NKV + (j + 1) * 128],
                   wvn[:, k * 1024 + half * 512:k * 1024 + (half + 1) * 512],
                   start=(k == 0), stop=(k == NT_E - 1))
            nc.vector.tensor_copy(
                vr[:, half * 8:(half + 1) * 8, 0:DHEAD],
                ps[:].rearrange("p (h d) -> p h d", d=DHEAD))

    for th in range(8):
        h0, h1 = 2 * th, 2 * th + 1
        if th < 7:
            emit_proj(th + 1)
        emit_pv(h0)
        emit_pv(h1)
        if th < 7:
            emit_qk(h0 + 2)
            emit_qk(h1 + 2)
        emit_t(th)

    # ---- final projection: out = O @ wo + bo ----
    for q_i in range(NT_Q):
        fin = pfin.tile([128, EMB], F32, tag="fin", name=f"fin{q_i}")
        for ch in range(2):
            pf = pps.tile([128, 512], F32, tag="ps", name=f"op{q_i}_{ch}")
            for k in range(NT_E):
                mm(pf[:], ot[k][:, q_i * 128:(q_i + 1) * 128],
                   won[:, k * 1024 + ch * 512:k * 1024 + (ch + 1) * 512],
                   start=(k == 0), stop=(k == NT_E - 1))
            nc.vector.tensor_add(fin[:, ch * 512:(ch + 1) * 512], pf[:],
                                 bob[:, ch * 512:(ch + 1) * 512])
            nc.sync.dma_start(
                aps["out"][q_i * 128:(q_i + 1) * 128, ch * 512:(ch + 1) * 512],
                fin[:, ch * 512:(ch + 1) * 512])

    if "qt_dbg" in aps:
        for th in range(8):
            nc.sync.dma_start(aps["qt_dbg"][:, th * 512:(th + 1) * 512],
                              qt[th][:])
            nc.sync.dma_start(aps["kt_dbg"][:, th * 768:(th + 1) * 768],
                              kt[th][:])
            nc.sync.dma_start(aps["ot_dbg"][:, th * 512:(th + 1) * 512],
                              ot[th][:])
        for j in range(6):
            nc.sync.dma_start(aps["v_dbg"][:, j * 1040:(j + 1) * 1040],
                              v[j][:])


_NC_CACHE = {}


def _build_nc():
    if "nc" in _NC_CACHE:
        return _NC_CACHE["nc"]
    nc = bacc.Bacc("TRN2", target_bir_lowering=False, debug=False,
                   enable_asserts=False, num_devices=8)
    aps = {}
    for name, shape, dt_ in [("xb", [128, NT_E * NKV], BF16),
                             ("wq", [128, 8192], BF16),
                             ("wk", [128, 8192], BF16),
                             ("wv", [128, 8192], BF16),
                             ("wo", [128, 8192], BF16),
                             ("mid", [128, 9 * 128], BF16),
                             ("bqc", [128, NT_E], F32),
                             ("bkc", [128, NT_E], F32),
                             ("bvc", [128, NT_E], F32),
                             ("bob", [128, EMB], F32)]:
        aps[name] = nc.dram_tensor(name, shape, dt_, kind="ExternalInput").ap()
    aps["out"] = nc.dram_tensor("out", [NQ, EMB], F32,
                                kind="ExternalOutput").ap()
    import os as _os
    if _os.environ.get("KERNEL_DEBUG") == "1":
        for nm, shape in [("qt_dbg", [128, 8 * 512]), ("kt_dbg", [128, 8 * 768]),
                          ("ot_dbg", [128, 8 * 512]), ("v_dbg", [128, 6 * 1040])]:
            aps[nm] = nc.dram_tensor(nm, shape, BF16,
                                     kind="ExternalOutput").ap()
    with tile.TileContext(nc) as tc:
        with ExitStack() as ctx:
            _body(ctx, tc, aps)
    nc.compile()
    _NC_CACHE["nc"] = nc
    return nc


def _kmajor(w):
    # [E, E] -> [128, 8*1024]: block k holds w[k*128:(k+1)*128, :]
    return np.ascontiguousarray(
        w.reshape(8, 128, EMB).transpose(1, 0, 2).reshape(128, 8 * EMB)
    ).astype(BF16_NP)


def _tmajor(w):
    # [E, E] -> [128, (t, k) blocks of 128]: col = t*1024 + k*128 + c
    return np.ascontiguousarray(
        w.reshape(8, 128, 8, 128).transpose(1, 2, 0, 3).reshape(128, 8192)
    ).astype(BF16_NP)


def _shard_inputs(x, wq, bq, wk, bk, wv, bv, wo, bo):
    x = np.asarray(x, dtype=np.float32)
    arrs = {
        "wq": _tmajor(np.asarray(wq, dtype=np.float32)),
        "wk": _tmajor(np.asarray(wk, dtype=np.float32)),
        "wv": _kmajor(np.asarray(wv, dtype=np.float32)),
        "wo": _kmajor(np.asarray(wo, dtype=np.float32)),
    }
    bq, bk, bv, bo = (np.asarray(b, dtype=np.float32) for b in (bq, bk, bv, bo))
    arrs["bqc"] = np.ascontiguousarray(bq.reshape(NT_E, 128).T)
    arrs["bkc"] = np.ascontiguousarray(bk.reshape(NT_E, 128).T)
    arrs["bvc"] = np.ascontiguousarray(bv.reshape(NT_E, 128).T)
    arrs["bob"] = np.ascontiguousarray(np.broadcast_to(bo, (128, EMB)))
    in_maps = []
    ar = np.arange(128)
    for core in range(8):
        b, c = core // 4, core % 4
        qs = c * NQ
        k0 = qs - QOFF  # first kv row; may be out of range (zero-padded)
        m = dict(arrs)
        xt = np.zeros((NKV, EMB), dtype=np.float32)
        lo, hi = max(0, k0), min(SEQ, k0 + NKV)
        xt[lo - k0:hi - k0, :] = x[b, lo:hi, :]
        m["xb"] = np.ascontiguousarray(
            xt.T.reshape(8, 128, NKV).transpose(1, 0, 2).reshape(128, 8 * NKV)
        ).astype(BF16_NP)
        mid = np.zeros((128, 9 * 128), dtype=np.float32)
        for t in range(4):
            qpos = qs + t * 128 + ar[None, :]
            for col, j in ((t, t), (4 + t, t + 2)):  # L then U block
                kpos = k0 + j * 128 + ar[:, None]
                mid[:, col * 128:(col + 1) * 128] = (
                    (np.abs(kpos - qpos) <= WIN) & (kpos >= 0) & (kpos < SEQ))
        mid[:, 8 * 128:] = np.eye(128, dtype=np.float32)
        m["mid"] = mid.astype(BF16_NP)
        in_maps.append(m)
    return in_maps


def kernel(x, wq, bq, wk, bk, wv, bv, wo, bo):
    global LAST_RESULT
    nc = _build_nc()
    in_maps = _shard_inputs(x, wq, bq, wk, bk, wv, bv, wo, bo)
    res = run_bass_kernel_spmd(nc, in_maps, core_ids=list(range(8)))
    LAST_RESULT = res
    out = np.empty((BATCH, SEQ, EMB), dtype=np.float32)
    for core in range(8):
        b, c = core // 4, core % 4
        out[b, c * NQ:(c + 1) * NQ, :] = res.results[core]["out"]
    return out


# revision 14
# speedup vs baseline: 1.0289x; 1.0289x over previous
"""LocalAttention Trainium2 Bass kernel (bf16, banded, flipped-PV).

Problem: x (2, 2048, 1024) f32 -> QKV proj (16 heads, d=64), local-window
attention (|i-j| <= 128), output projection.

Sharding (no collectives): 8 shards = 2 batches x 4 seq-chunks of 512 q rows.
Each core gets a 768-row KV halo slice [qs-128, qs+640), zero-padded at the
sequence edges, so q row i sits at kv row i+128 on every core.

Attention is banded with 128-row q tiles: q tile t's window covers exactly kv
tiles t, t+1, t+2 of the local 6-tile kv range. Tile t+1 is always fully
in-window (no mask); tiles t and t+2 get canonical triangular masks (edge
validity folded in per core); the remaining 3 kv tiles are skipped entirely.

Per (head, q-tile) the PV matmul runs "flipped" -- out [128 q, 64 v + 1 ones]
-- so the softmax denominator lands per-partition: a [128,1] reciprocal and a
per-partition tensor_scalar_mul normalize it. Head pairs are then transposed
back to [head*64, q] via the PE (identity matmul) for the output projection.

All matmul operands are bf16 (f32 PSUM accumulation). Engine split: PE
matmuls/transposes; Scalar exp only; GpSimd proj copies + half the mask muls;
Vector the rest. The emission order software-pipelines head pairs so the PE
stays continuously busy (p-state ramp) while Scalar/Vector chew on the
previous head's softmax.
"""

from contextlib import ExitStack

import numpy as np
import ml_dtypes

import concourse.bacc as bacc
import concourse.mybir as mybir
import concourse.tile as tile
from concourse.bass_utils import run_bass_kernel_spmd

F32 = mybir.dt.float32
BF16 = mybir.dt.bfloat16
AF = mybir.ActivationFunctionType
BF16_NP = ml_dtypes.bfloat16

EMB = 1024
NHEAD = 16
DHEAD = 64
WIN = 128
BATCH = 2
SEQ = 2048
NQ = 512            # q rows per core
NKV = 768           # kv rows per core (q rows + clamped 128 halo each side)
NT_E = EMB // 128   # 8 e-tiles
NT_KV = NKV // 128  # 6 kv-tiles
NT_Q = NQ // 128    # 4 q-tiles of 128
QOFF = 128          # q row i == kv row i + QOFF
SCALE = 1.0 / np.sqrt(EMB)

LAST_RESULT = None  # BassKernelResults of the most recent run (for profiling)


def _body(ctx, tc, aps):
    nc = tc.nc

    pbig = ctx.enter_context(tc.tile_pool(name="big", bufs=1))
    pqt = ctx.enter_context(tc.tile_pool(name="qt", bufs=3))
    pkt = ctx.enter_context(tc.tile_pool(name="kt", bufs=3))
    pv = ctx.enter_context(tc.tile_pool(name="v", bufs=6))
    pset = ctx.enter_context(tc.tile_pool(name="set", bufs=12))
    pov = ctx.enter_context(tc.tile_pool(name="ov", bufs=4))
    pot = ctx.enter_context(tc.tile_pool(name="ot", bufs=8))
    pfin = ctx.enter_context(tc.tile_pool(name="fin", bufs=2))
    psml = ctx.enter_context(tc.tile_pool(name="small", bufs=1))
    prc = ctx.enter_context(tc.tile_pool(name="rc", bufs=6))
    pps = ctx.enter_context(tc.tile_pool(name="ps", bufs=3, space="PSUM"))
    pqk = ctx.enter_context(tc.tile_pool(name="qk", bufs=2, space="PSUM"))
    ppv = ctx.enter_context(tc.tile_pool(name="pv", bufs=2, space="PSUM"))
    ppt = ctx.enter_context(tc.tile_pool(name="pt", bufs=1, space="PSUM"))

    mm = nc.tensor.matmul

    # ---- persistent tiles ----
    xb = pbig.tile([128, NT_E * NKV], BF16, tag="xb")
    wqt = pbig.tile([128, EMB * NT_E // 8 * 8], BF16, tag="wq", name="wqt")
    wkt = pbig.tile([128, 8192], BF16, tag="wk", name="wkt")
    wvn = pbig.tile([128, 8192], BF16, tag="wv", name="wvn")
    won = pbig.tile([128, 8192], BF16, tag="wo", name="won")
    mid = pbig.tile([128, 9 * 128], BF16, tag="mid", name="mid")
    bqs = psml.tile([128, NT_E], F32, tag="bq", name="bqs")
    bks = psml.tile([128, NT_E], F32, tag="bk", name="bks")
    bvs = psml.tile([128, NT_E], F32, tag="bv", name="bvs")
    bob = psml.tile([128, EMB], F32, tag="bo", name="bob")
    wrm = psml.tile([1, 1], F32, tag="wrm", name="wrm")

    # warm the Exp activation table before the first real softmax needs it
    nc.gpsimd.memset(wrm[:], 0.0)
    nc.scalar.activation(wrm[:], wrm[:], AF.Exp, scale=1.0)

    # ---- input DMAs ----
    # Critical path first: Q-proj(0) needs wq block 0 + all x blocks. Dispatch
    # wq0/wk0 on the GpSimd DGE queue (in parallel with x on Sync) so the
    # first matmul can start early; everything else streams behind x.
    nc.gpsimd.dma_start(wqt[:, 0:1024], aps["wq"][:, 0:1024])
    nc.gpsimd.dma_start(wkt[:, 0:1024], aps["wk"][:, 0:1024])
    for k in range(NT_E):  # x blocks split across two DGE queues
        eng = nc.sync if k % 2 == 0 else nc.scalar
        eng.dma_start(xb[:, k * NKV:(k + 1) * NKV],
                      aps["xb"][:, k * NKV:(k + 1) * NKV])
    nc.scalar.dma_start(mid[:], aps["mid"][:])
    nc.scalar.dma_start(bqs[:], aps["bqc"][:])
    nc.scalar.dma_start(bks[:], aps["bkc"][:])
    nc.scalar.dma_start(bvs[:], aps["bvc"][:])
    nc.scalar.dma_start(bob[:], aps["bob"][:])
    for k in range(NT_E):
        nc.sync.dma_start(wvn[:, k * 1024:(k + 1) * 1024],
                          aps["wv"][:, k * 1024:(k + 1) * 1024])
    for t in range(1, 8):
        nc.sync.dma_start(wqt[:, t * 1024:(t + 1) * 1024],
                          aps["wq"][:, t * 1024:(t + 1) * 1024])
        nc.sync.dma_start(wkt[:, t * 1024:(t + 1) * 1024],
                          aps["wk"][:, t * 1024:(t + 1) * 1024])
    for k in range(NT_E):
        nc.sync.dma_start(won[:, k * 1024:(k + 1) * 1024],
                          aps["wo"][:, k * 1024:(k + 1) * 1024])

    ident = mid[:, 8 * 128:9 * 128]

    # ---- v tiles (with ones column per head for the softmax denominator) ----
    v = []
    for j in range(NT_KV):
        vt = pv.tile([128, NHEAD * (DHEAD + 1)], BF16, tag="v", name=f"v{j}")
        vr = vt[:].rearrange("p (h d) -> p h d", d=DHEAD + 1)
        nc.gpsimd.memset(vr[:, :, DHEAD:DHEAD + 1], 1.0)
        v.append(vt)

    qt = [None] * NT_E
    kt = [None] * NT_E
    ot = [pot.tile([128, NQ], BF16, tag="ot", name=f"ot{i}") for i in range(NT_E)]
    sets = {}  # h -> list of (set_tile, tmin) per kv tile j
    ovs = {}   # t -> ov tile for the current head pair

    def emit_proj(th):
        # wq/wk are t-major: block th holds [k, 128] col-tiles contiguously.
        ps = pps.tile([128, NQ], F32, tag="ps", name=f"qp{th}")
        for k in range(NT_E):
            mm(ps[:], wqt[:, th * 1024 + k * 128:th * 1024 + (k + 1) * 128],
               xb[:, k * NKV + QOFF:k * NKV + QOFF + NQ],
               start=(k == 0), stop=(k == NT_E - 1))
        qt[th] = pqt.tile([128, NQ], BF16, tag="qt", name=f"qt{th}")
        nc.scalar.activation(qt[th][:], ps[:], AF.Identity,
                             bias=bqs[:, th:th + 1])
        kt[th] = pkt.tile([128, NKV], BF16, tag="kt", name=f"kt{th}")
        for half in range(2):
            ps = pps.tile([128, NQ], F32, tag="ps", name=f"kp{th}_{half}")
            for k in range(NT_E):
                mm(ps[:, 0:384],
                   wkt[:, th * 1024 + k * 128:th * 1024 + (k + 1) * 128],
                   xb[:, k * NKV + half * 384:k * NKV + (half + 1) * 384],
                   start=(k == 0), stop=(k == NT_E - 1))
            nc.vector.tensor_scalar_add(kt[th][:, half * 384:(half + 1) * 384],
                                        ps[:, 0:384], bks[:, th:th + 1])

    def emit_qk(h):
        th, off = h // 2, (h % 2) * DHEAD
        hs = []
        for j in range(NT_KV):
            tmin, tmax = max(0, j - 2), min(3, j)
            w = (tmax - tmin + 1) * 128
            qk = pqk.tile([128, 384], F32, tag="qk", name=f"qk{h}_{j}")
            mm(qk[:, 0:w], kt[th][off:off + DHEAD, j * 128:(j + 1) * 128],
               qt[th][off:off + DHEAD, tmin * 128:(tmax + 1) * 128],
               start=True, stop=True)
            st = pset.tile([128, 384], BF16, tag="set", name=f"set{h}_{j}")
            nc.scalar.activation(st[:, 0:w], qk[:, 0:w], AF.Exp,
                                 scale=float(SCALE))
            if j <= 3:  # lower-edge mask block at t == j
                eng = nc.gpsimd
                c0 = (j - tmin) * 128
                eng.tensor_mul(st[:, c0:c0 + 128], st[:, c0:c0 + 128],
                               mid[:, j * 128:(j + 1) * 128])
            if j >= 2:  # upper-edge mask block at t == j - 2
                eng = nc.gpsimd
                c0 = (j - 2 - tmin) * 128
                eng.tensor_mul(st[:, c0:c0 + 128], st[:, c0:c0 + 128],
                               mid[:, (j + 2) * 128:(j + 3) * 128])
            hs.append((st, tmin))
        sets[h] = hs

    def emit_pv(h):
        th, off = h // 2, (h % 2) * DHEAD
        for t in range(NT_Q):
            pvp = ppv.tile([128, DHEAD + 1], F32, tag="pv", name=f"pv{h}_{t}")
            for i, j in enumerate(range(t, t + 3)):
                st, tmin = sets[h][j]
                c0 = (t - tmin) * 128
                mm(pvp[:], st[:, c0:c0 + 128],
                   v[j][:, h * (DHEAD + 1):(h + 1) * (DHEAD + 1)],
                   start=(i == 0), stop=(i == 2))
            rc = prc.tile([128, 1], F32, tag="rc", name=f"rc{h}_{t}")
            nc.vector.reciprocal(rc[:], pvp[:, DHEAD:DHEAD + 1])
            if h % 2 == 0:
                ovs[t] = pov.tile([128, 128], BF16, tag="ov", name=f"ov{h}_{t}")
            nc.vector.tensor_scalar_mul(ovs[t][:, off:off + DHEAD],
                                        pvp[:, 0:DHEAD], rc[:])
        del sets[h]

    def emit_t(th):
        tp = ppt.tile([128, 512], BF16, tag="tp", name=f"tp{th}")
        for t in range(NT_Q):
            nc.tensor.transpose(tp[:, t * 128:(t + 1) * 128], ovs[t][:], ident)
        nc.vector.tensor_scalar_add(ot[th][:], tp[:], bvs[:, th:th + 1])

    # ---- emission: prologue, then the software-pipelined head-pair loop ----
    emit_proj(0)
    emit_qk(0)
    emit_qk(1)

    for j in range(NT_KV):  # V projection (kv-tile major)
        vr = v[j][:].rearrange("p (h d) -> p h d", d=DHEAD + 1)
        for half in range(2):
            ps = pps.tile([128, NQ], F32, tag="ps", name=f"vp{j}_{half}")
            for k in range(NT_E):
                mm(ps[:], xb[:, k * NKV + j * 128:k * NKV + (j + 1) * 128],
                   wvn[:, k * 1024 + half * 512:k * 1024 + (half + 1) * 512],
                   start=(k == 0), stop=(k == NT_E - 1))
            nc.vector.tensor_copy(
                vr[:, half * 8:(half + 1) * 8, 0:DHEAD],
                ps[:].rearrange("p (h d) -> p h d", d=DHEAD))

    for th in range(8):
        h0, h1 = 2 * th, 2 * th + 1
        if th < 7:
            emit_proj(th + 1)
        emit_pv(h0)
        emit_pv(h1)
        if th < 7:
            emit_qk(h0 + 2)
            emit_qk(h1 + 2)
        emit_t(th)

    # ---- final projection: out = O @ wo + bo ----
    for q_i in range(NT_Q):
        fin = pfin.tile([128, EMB], F32, tag="fin", name=f"fin{q_i}")
        for ch in range(2):
            pf = pps.tile([128, 512], F32, tag="ps", name=f"op{q_i}_{ch}")
            for k in range(NT_E):
                mm(pf[:], ot[k][:, q_i * 128:(q_i + 1) * 128],
                   won[:, k * 1024 + ch * 512:k * 1024 + (ch + 1) * 512],
                   start=(k == 0), stop=(k == NT_E - 1))
            nc.vector.tensor_add(fin[:, ch * 512:(ch + 1) * 512], pf[:],
                                 bob[:, ch * 512:(ch + 1) * 512])
            nc.sync.dma_start(
                aps["out"][q_i * 128:(q_i + 1) * 128, ch * 512:(ch + 1) * 512],
                fin[:, ch * 512:(ch + 1) * 512])

    if "qt_dbg" in aps:
        for th in range(8):
            nc.sync.dma_start(aps["qt_dbg"][:, th * 512:(th + 1) * 512],
                              qt[th][:])
            nc.sync.dma_start(aps["kt_dbg"][:, th * 768:(th + 1) * 768],
                              kt[th][:])
            nc.sync.dma_start(aps["ot_dbg"][:, th * 512:(th + 1) * 512],
                              ot[th][:])
        for j in range(6):
            nc.sync.dma_start(aps["v_dbg"][:, j * 1040:(j + 1) * 1040],
                              v[j][:])


_NC_CACHE = {}


def _build_nc():
    if "nc" in _NC_CACHE:
        return _NC_CACHE["nc"]
    nc = bacc.Bacc("TRN2", target_bir_lowering=False, debug=False,
                   enable_asserts=False, num_devices=8)
    aps = {}
    for name, shape, dt_ in [("xb", [128, NT_E * NKV], BF16),
                             ("wq", [128, 8192], BF16),
                             ("wk", [128, 8192], BF16),
                             ("wv", [128, 8192], BF16),
                             ("wo", [128, 8192], BF16),
                             ("mid", [128, 9 * 128], BF16),
                             ("bqc", [128, NT_E], F32),
                             ("bkc", [128, NT_E], F32),
                             ("bvc", [128, NT_E], F32),
                             ("bob", [128, EMB], F32)]:
        aps[name] = nc.dram_tensor(name, shape, dt_, kind="ExternalInput").ap()
    aps["out"] = nc.dram_tensor("out", [NQ, EMB], F32,
                                kind="ExternalOutput").ap()
    import os as _os
    if _os.environ.get("KERNEL_DEBUG") == "1":
        for nm, shape in [("qt_dbg", [128, 8 * 512]), ("kt_dbg", [128, 8 * 768]),
                          ("ot_dbg", [128, 8 * 512]), ("v_dbg", [128, 6 * 1040])]:
            aps[nm] = nc.dram_tensor(nm, shape, BF16,
                                     kind="ExternalOutput").ap()
    with tile.TileContext(nc) as tc:
        with ExitStack() as ctx:
            _body(ctx, tc, aps)
    nc.compile()
    _NC_CACHE["nc"] = nc
    return nc


def _kmajor(w):
    # [E, E] -> [128, 8*1024]: block k holds w[k*128:(k+1)*128, :]
    return np.ascontiguousarray(
        w.reshape(8, 128, EMB).transpose(1, 0, 2).reshape(128, 8 * EMB)
    ).astype(BF16_NP)


def _tmajor(w):
    # [E, E] -> [128, (t, k) blocks of 128]: col = t*1024 + k*128 + c
    return np.ascontiguousarray(
        w.reshape(8, 128, 8, 128).transpose(1, 2, 0, 3).reshape(128, 8192)
    ).astype(BF16_NP)


def _shard_inputs(x, wq, bq, wk, bk, wv, bv, wo, bo):
    x = np.asarray(x, dtype=np.float32)
    arrs = {
        "wq": _tmajor(np.asarray(wq, dtype=np.float32)),
        "wk": _tmajor(np.asarray(wk, dtype=np.float32)),
        "wv": _kmajor(np.asarray(wv, dtype=np.float32)),
        "wo": _kmajor(np.asarray(wo, dtype=np.float32)),
    }
    bq, bk, bv, bo = (np.asarray(b, dtype=np.float32) for b in (bq, bk, bv, bo))
    arrs["bqc"] = np.ascontiguousarray(bq.reshape(NT_E, 128).T)
    arrs["bkc"] = np.ascontiguousarray(bk.reshape(NT_E, 128).T)
    arrs["bvc"] = np.ascontiguousarray(bv.reshape(NT_E, 128).T)
    arrs["bob"] = np.ascontiguousarray(np.broadcast_to(bo, (128, EMB)))
    in_maps = []
    ar = np.arange(128)
    for core in range(8):
        b, c = core // 4, core % 4
        qs = c * NQ
        k0 = qs - QOFF  # first kv row; may be out of range (zero-padded)
        m = dict(arrs)
        xt = np.zeros((NKV, EMB), dtype=np.float32)
        lo, hi = max(0, k0), min(SEQ, k0 + NKV)
        xt[lo - k0:hi - k0, :] = x[b, lo:hi, :]
        m["xb"] = np.ascontiguousarray(
            xt.T.reshape(8, 128, NKV).transpose(1, 0, 2).reshape(128, 8 * NKV)
        ).astype(BF16_NP)
        mid = np.zeros((128, 9 * 128), dtype=np.float32)
        for t in range(4):
            qpos = qs + t * 128 + ar[None, :]
            for col, j in ((t, t), (4 + t, t + 2)):  # L then U block
                kpos = k0 + j * 128 + ar[:, None]
                mid[:, col * 128:(col + 1) * 128] = (
                    (np.abs(kpos - qpos) <= WIN) & (kpos >= 0) & (kpos < SEQ))
        mid[:, 8 * 128:] = np.eye(128, dtype=np.float32)
        m["mid"] = mid.astype(BF16_NP)
        in_maps.append(m)
    return in_maps


def kernel(x, wq, bq, wk, bk, wv, bv, wo, bo):
    global LAST_RESULT
    nc = _build_nc()
    in_maps = _shard_inputs(x, wq, bq, wk, bk, wv, bv, wo, bo)
    res = run_bass_kernel_spmd(nc, in_maps, core_ids=list(range(8)))
    LAST_RESULT = res
    out = np.empty((BATCH, SEQ, EMB), dtype=np.float32)
    for core in range(8):
        b, c = core // 4, core % 4
        out[b, c * NQ:(c + 1) * NQ, :] = res.results[core]["out"]
    return out
